# revision 5
# baseline (speedup 1.0000x reference)
"""Trainium2 Bass kernel for masked causal depthwise Conv1D (prefill path).

Reference computation (fp32):
    doc_mask = (segment_pos != 0)                       # [B, S]
    xm = x * doc_mask[..., None]                        # [B, S, W]
    out[b, t, c] = b[c] + sum_k w[k, c] * xm[b, t-3+k, c]   (causal, TW=4)
    new_cache = xm[:, -3:]                              # [B, 3, W]

Shapes: x [4, 4096, 2560] f32, segment_pos [4, 4096] i32, w [4, 2560] f32,
b [2560] f32.

Strategy (8 NeuronCores):
  Shard by (batch, channel-half): core i <- batch i//2, channels
  [1280*(i%2), 1280*(i%2+1)).  Each core receives its x shard transposed to
  channel-major [1280, 4099] (3 leading zero columns = causal pad), so both
  the DMA loads and stores are contiguous per partition and the conv taps
  become free-dim column offsets.

  Per 128-channel block (10 per core):
    - mask:  one fused DVE scalar_tensor_tensor:  xm = (spos != 0) * x
             (spos broadcast across partitions once per core via log2 DMAs)
    - taps:  4 PE matmuls accumulating in PSUM: lhsT = diag(w_k) (static,
             host-built), rhs = xm[:, j*512+k : +512], so
             psum[c, t] += w_k[c] * xm[c, t+k].  float32r streams at
             1 cycle/row (fp32 would be 4x slower).
    - evict: ACT Identity with per-partition bias AP: out = psum + b[c]
    - DMA out [128, 4096] + the last-3-column masked slice (new_cache).

Host does only layout work (transpose/pad/diag-embed/slice); all arithmetic
on the 41.9M-element tensors runs on device.
"""

import numpy as np

B, S, W, TW = 4, 4096, 2560, 4
N_CORES = 8
CH = W // 2          # 1280 channels per core
CBS = CH // 128      # 10 channel blocks per core
PAD = TW - 1         # 3
SP = S + PAD         # 4099 padded time columns
NT = S // 512        # 8 psum tiles of 512 per channel block

# matmul dtype: "f32r" = float32r (full-rate, slightly reduced multiply
# precision), "f32" = native fp32 (4 cycles/row, bit-accurate)
MATMUL_DTYPE = "f32r"

_PROGRAM_CACHE: dict = {}


def _build_program(reps: int = 1, matmul_dtype: str = MATMUL_DTYPE):
    """Build the per-core Bass program. All 8 cores run this SPMD with
    different data. reps>1 repeats the whole compute (for timing)."""
    import concourse.bass as bass
    import concourse.tile as tile
    import concourse.mybir as mybir
    from concourse import bacc

    key = (reps, matmul_dtype)
    if key in _PROGRAM_CACHE:
        return _PROGRAM_CACHE[key]

    f32 = mybir.dt.float32
    f32r = mybir.dt.float32r
    i32 = mybir.dt.int32
    mm_dt = f32r if matmul_dtype == "f32r" else f32

    nc = bacc.Bacc("TRN2", target_bir_lowering=False, debug=False,
                   num_devices=N_CORES)

    xp_d = nc.dram_tensor("xp", [CH, SP], f32, kind="ExternalInput")
    spos_d = nc.dram_tensor("spos", [1, S], i32, kind="ExternalInput")
    wdiag_d = nc.dram_tensor("wdiag", [CBS, TW, 128, 128], mm_dt,
                             kind="ExternalInput")
    biasr_d = nc.dram_tensor("biasr", [128, CBS], f32, kind="ExternalInput")
    out_d = nc.dram_tensor("outT", [CH, S], f32, kind="ExternalOutput")
    cache_d = nc.dram_tensor("cacheT", [CH, PAD], f32, kind="ExternalOutput")

    with tile.TileContext(nc) as tc:
        with (
            tc.tile_pool(name="const", bufs=1) as const_pool,
            tc.tile_pool(name="xin", bufs=2) as x_pool,
            tc.tile_pool(name="xmask", bufs=2) as xm_pool,
            tc.tile_pool(name="outp", bufs=2) as out_pool,
            tc.tile_pool(name="psum", bufs=8, space="PSUM") as psum_pool,
        ):
            # --- constants, loaded once ---
            wdiag_t = const_pool.tile([128, CBS * TW * 128], mm_dt)
            for cb in range(CBS):
                for k in range(TW):
                    idx = cb * TW + k
                    nc.sync.dma_start(
                        wdiag_t[:, idx * 128:(idx + 1) * 128],
                        wdiag_d[cb, k])
            biasr_t = const_pool.tile([128, CBS], f32)
            nc.sync.dma_start(biasr_t[:], biasr_d[:])

            # segment_pos broadcast to all 128 partitions, padded to SP cols.
            # Pad cols get 1 (-> mask 1.0; they multiply the zero pad of x).
            spos_bc = const_pool.tile([128, SP], i32)
            nc.vector.memset(spos_bc[0:1, 0:PAD], 1)
            nc.sync.dma_start(spos_bc[0:1, PAD:SP], spos_d[:])
            p = 1
            while p < 128:
                nc.sync.dma_start(spos_bc[p:2 * p, :], spos_bc[0:p, :])
                p *= 2

            for _ in range(reps):
                for cb in range(CBS):
                    xt = x_pool.tile([128, SP], f32)
                    nc.sync.dma_start(xt[:], xp_d[cb * 128:(cb + 1) * 128, :])

                    # xm = (spos != 0) * x   — one fused DVE op; output is
                    # written in the matmul streaming dtype (f32r rounds)
                    xm = xm_pool.tile([128, SP], mm_dt)
                    nc.vector.scalar_tensor_tensor(
                        xm[:], spos_bc[:], 0.0, xt[:],
                        op0=mybir.AluOpType.not_equal,
                        op1=mybir.AluOpType.mult)

                    # exact f32 masked copy of the last TW-1 timesteps
                    cc = out_pool.tile([128, PAD], f32, tag="cachecol")
                    nc.vector.scalar_tensor_tensor(
                        cc[:], spos_bc[:, S:SP], 0.0, xt[:, S:SP],
                        op0=mybir.AluOpType.not_equal,
                        op1=mybir.AluOpType.mult)

                    ot = out_pool.tile([128, S], f32)
                    for j in range(NT):
                        ps = psum_pool.tile([128, 512], f32)
                        for k in range(TW):
                            idx = cb * TW + k
                            nc.tensor.matmul(
                                ps[:],
                                wdiag_t[:, idx * 128:(idx + 1) * 128],
                                xm[:, j * 512 + k: j * 512 + k + 512],
                                start=(k == 0), stop=(k == TW - 1))
                        # out = psum + bias[c]  (per-partition bias AP)
                        nc.scalar.activation(
                            ot[:, j * 512:(j + 1) * 512], ps[:],
                            mybir.ActivationFunctionType.Identity,
                            bias=biasr_t[:, cb:cb + 1], scale=1.0)

                    nc.sync.dma_start(out_d[cb * 128:(cb + 1) * 128, :], ot[:])
                    nc.sync.dma_start(cache_d[cb * 128:(cb + 1) * 128, :],
                                      cc[:])

    nc.compile()
    _PROGRAM_CACHE[key] = nc
    return nc


def _make_in_maps(x, segment_pos, w, b):
    """Host-side sharding: pure layout work (transpose, pad, diag-embed)."""
    x = np.asarray(x, dtype=np.float32)
    segment_pos = np.ascontiguousarray(np.asarray(segment_pos, dtype=np.int32))
    w = np.asarray(w, dtype=np.float32)
    b = np.asarray(b, dtype=np.float32)

    # [B, W, S] channel-major copy once
    xT = np.ascontiguousarray(x.transpose(0, 2, 1))

    halves = []
    for h in range(2):
        wd = np.zeros((CBS, TW, 128, 128), dtype=np.float32)
        for cb in range(CBS):
            for k in range(TW):
                np.fill_diagonal(wd[cb, k],
                                 w[k, h * CH + cb * 128: h * CH + (cb + 1) * 128])
        br = np.ascontiguousarray(
            b[h * CH:(h + 1) * CH].reshape(CBS, 128).T)
        halves.append((wd, br))

    in_maps = []
    for i in range(N_CORES):
        bi, h = divmod(i, 2)
        xp = np.zeros((CH, SP), dtype=np.float32)
        xp[:, PAD:] = xT[bi, h * CH:(h + 1) * CH, :]
        wd, br = halves[h]
        in_maps.append({
            "xp": xp,
            "spos": segment_pos[bi:bi + 1],
            "wdiag": wd,
            "biasr": br,
        })
    return in_maps


def _assemble(results):
    out = np.empty((B, S, W), dtype=np.float32)
    cache = np.empty((B, PAD, W), dtype=np.float32)
    for i, r in enumerate(results):
        bi, h = divmod(i, 2)
        out[bi, :, h * CH:(h + 1) * CH] = r["outT"].T
        cache[bi, :, h * CH:(h + 1) * CH] = r["cacheT"].T
    return out, cache


def run_on_device(in_maps, reps: int = 1, matmul_dtype: str = MATMUL_DTYPE):
    from concourse.bass_utils import run_bass_kernel_spmd
    nc = _build_program(reps, matmul_dtype)
    res = run_bass_kernel_spmd(nc, in_maps, core_ids=list(range(N_CORES)))
    return res.results


def kernel(x, segment_pos, w, b):
    in_maps = _make_in_maps(x, segment_pos, w, b)
    results = run_on_device(in_maps)
    return _assemble(results)


# revision 8
# speedup vs baseline: 107.7680x; 107.7680x over previous
"""Trainium2 Bass kernel for masked causal depthwise Conv1D (prefill path).

Reference computation (fp32):
    doc_mask = (segment_pos != 0)                       # [B, S]
    xm = x * doc_mask[..., None]                        # [B, S, W]
    out[b, t, c] = b[c] + sum_k w[k, c] * xm[b, t-3+k, c]   (causal, TW=4)
    new_cache = xm[:, -3:]                              # [B, 3, W]

Shapes: x [4, 4096, 2560] f32, segment_pos [4, 4096] i32, w [4, 2560] f32,
b [2560] f32.

Strategy (8 NeuronCores):
  Shard by (batch, channel-half): core i <- batch i//2, channels
  [1280*(i%2), 1280*(i%2+1)).  Each core receives its x shard transposed to
  channel-major [1280, 4099] (3 leading zero columns = causal pad), so both
  the DMA loads and stores are contiguous per partition and the conv taps
  become free-dim column offsets.

  Per 128-channel block (10 per core):
    - mask:  one fused DVE scalar_tensor_tensor:  xm = (spos != 0) * x
             (spos broadcast across partitions by one step-0 DMA)
    - taps:  4 PE matmuls accumulating in PSUM: lhsT = diag(w_k) (static,
             host-built), rhs = xm[:, j*512+k : +512], so
             psum[c, t] += w_k[c] * xm[c, t+k-3].  float32r streams at
             1 cycle/row (fp32 would be 4x slower).
    - evict: ACT Identity with per-partition bias AP: out = psum + b[c]
  Loads run on the SP HWDGE ring (nc.sync), stores on the ACT ring
  (nc.scalar) so the two directions don't serialize on one FIFO.

Host does only layout work (transpose/pad/diag-embed/slice); all arithmetic
on the 41.9M-element tensors runs on device.
"""

import numpy as np

B, S, W, TW = 4, 4096, 2560, 4
N_CORES = 8
CH = W // 2          # 1280 channels per core
CBS = CH // 128      # 10 channel blocks per core
PAD = TW - 1         # 3
SP = S + PAD         # 4099 padded time columns
NT = S // 512        # 8 psum tiles of 512 per channel block

# matmul dtype: "f32r" = float32r (full-rate, slightly reduced multiply
# precision), "f32" = native fp32 (4 cycles/row, bit-accurate)
MATMUL_DTYPE = "f32r"

_PROGRAM_CACHE: dict = {}


def _build_program(reps: int = 1, matmul_dtype: str = MATMUL_DTYPE):
    """Build the per-core Bass program. All 8 cores run this SPMD with
    different data. reps>1 repeats the whole compute (for timing)."""
    import concourse.bass as bass
    import concourse.tile as tile
    import concourse.mybir as mybir
    from concourse import bacc

    key = (reps, matmul_dtype)
    if key in _PROGRAM_CACHE:
        return _PROGRAM_CACHE[key]

    f32 = mybir.dt.float32
    i32 = mybir.dt.int32
    mm_dt = mybir.dt.float32r if matmul_dtype == "f32r" else f32

    nc = bacc.Bacc("TRN2", target_bir_lowering=False, debug=False,
                   num_devices=N_CORES)

    xp_d = nc.dram_tensor("xp", [CH, SP], f32, kind="ExternalInput")
    spos_d = nc.dram_tensor("spos", [1, S], i32, kind="ExternalInput")
    # host-prebuilt diagonal embedding of w, laid out [p, (cb, k, m)]
    wdiag_d = nc.dram_tensor("wdiag", [128, CBS * TW * 128], mm_dt,
                             kind="ExternalInput")
    biasr_d = nc.dram_tensor("biasr", [128, CBS], f32, kind="ExternalInput")
    out_d = nc.dram_tensor("outT", [CH, S], f32, kind="ExternalOutput")
    # cache laid out [p, (cb, PAD)]; host un-permutes
    cache_d = nc.dram_tensor("cacheT", [128, CBS * PAD], f32,
                             kind="ExternalOutput")

    with tile.TileContext(nc) as tc:
        with (
            tc.tile_pool(name="const", bufs=1) as const_pool,
            tc.tile_pool(name="xin", bufs=2) as x_pool,
            tc.tile_pool(name="xmask", bufs=2) as xm_pool,
            tc.tile_pool(name="outp", bufs=2) as out_pool,
            tc.tile_pool(name="psum", bufs=8, space="PSUM") as psum_pool,
        ):
            # --- constants, loaded once ---
            wdiag_t = const_pool.tile([128, CBS * TW * 128], mm_dt)
            nc.sync.dma_start(wdiag_t[:], wdiag_d[:])
            biasr_t = const_pool.tile([128, CBS], f32)
            nc.sync.dma_start(biasr_t[:], biasr_d[:])

            # segment_pos broadcast to all 128 partitions, padded to SP cols.
            # Pad cols get 1 (-> mask 1.0; they multiply the zero pad of x).
            spos_bc = const_pool.tile([128, SP], i32)
            nc.vector.memset(spos_bc[:, 0:PAD], 1)
            nc.sync.dma_start(spos_bc[:, PAD:SP],
                              spos_d.ap().to_broadcast((128, S)))

            cache_t = const_pool.tile([128, CBS * PAD], f32)

            for rep in range(reps):
                for cb in range(CBS):
                    xt = x_pool.tile([128, SP], f32)
                    nc.sync.dma_start(xt[:], xp_d[cb * 128:(cb + 1) * 128, :])

                    # xm = (spos != 0) * x   — one fused DVE op; output is
                    # written in the matmul streaming dtype (f32r rounds)
                    xm = xm_pool.tile([128, SP], mm_dt)
                    nc.vector.scalar_tensor_tensor(
                        xm[:], spos_bc[:], 0.0, xt[:],
                        op0=mybir.AluOpType.not_equal,
                        op1=mybir.AluOpType.mult)

                    # exact f32 masked copy of the last TW-1 timesteps
                    nc.vector.scalar_tensor_tensor(
                        cache_t[:, cb * PAD:(cb + 1) * PAD],
                        spos_bc[:, S:SP], 0.0, xt[:, S:SP],
                        op0=mybir.AluOpType.not_equal,
                        op1=mybir.AluOpType.mult)

                    ot = out_pool.tile([128, S], f32)
                    for j in range(NT):
                        ps = psum_pool.tile([128, 512], f32)
                        for k in range(TW):
                            idx = cb * TW + k
                            nc.tensor.matmul(
                                ps[:],
                                wdiag_t[:, idx * 128:(idx + 1) * 128],
                                xm[:, j * 512 + k: j * 512 + k + 512],
                                start=(k == 0), stop=(k == TW - 1))
                        # out = psum + bias[c]  (per-partition bias AP)
                        nc.scalar.activation(
                            ot[:, j * 512:(j + 1) * 512], ps[:],
                            mybir.ActivationFunctionType.Identity,
                            bias=biasr_t[:, cb:cb + 1], scale=1.0)

                    nc.scalar.dma_start(out_d[cb * 128:(cb + 1) * 128, :],
                                        ot[:])
                nc.scalar.dma_start(cache_d[:], cache_t[:])

    nc.compile()
    _PROGRAM_CACHE[key] = nc
    return nc


def _make_in_maps(x, segment_pos, w, b):
    """Host-side sharding: pure layout work (transpose, pad, diag-embed)."""
    x = np.asarray(x, dtype=np.float32)
    segment_pos = np.ascontiguousarray(np.asarray(segment_pos, dtype=np.int32))
    w = np.asarray(w, dtype=np.float32)
    b = np.asarray(b, dtype=np.float32)

    # [B, W, S] channel-major copy once
    xT = np.ascontiguousarray(x.transpose(0, 2, 1))

    idx = np.arange(128)
    halves = []
    for h in range(2):
        wd = np.zeros((128, CBS, TW, 128), dtype=np.float32)
        for cb in range(CBS):
            for k in range(TW):
                wd[idx, cb, k, idx] = w[k, h * CH + cb * 128 + idx]
        wd = np.ascontiguousarray(wd.reshape(128, CBS * TW * 128))
        br = np.ascontiguousarray(
            b[h * CH:(h + 1) * CH].reshape(CBS, 128).T)
        halves.append((wd, br))

    in_maps = []
    for i in range(N_CORES):
        bi, h = divmod(i, 2)
        xp = np.zeros((CH, SP), dtype=np.float32)
        xp[:, PAD:] = xT[bi, h * CH:(h + 1) * CH, :]
        wd, br = halves[h]
        in_maps.append({
            "xp": xp,
            "spos": segment_pos[bi:bi + 1],
            "wdiag": wd,
            "biasr": br,
        })
    return in_maps


def _assemble(results):
    out = np.empty((B, S, W), dtype=np.float32)
    cache = np.empty((B, PAD, W), dtype=np.float32)
    for i, r in enumerate(results):
        bi, h = divmod(i, 2)
        out[bi, :, h * CH:(h + 1) * CH] = r["outT"].T
        # cacheT [128, CBS*PAD] -> [CH, PAD]
        ct = r["cacheT"].reshape(128, CBS, PAD).transpose(1, 0, 2).reshape(CH, PAD)
        cache[bi, :, h * CH:(h + 1) * CH] = ct.T
    return out, cache


def run_on_device(in_maps, reps: int = 1, matmul_dtype: str = MATMUL_DTYPE):
    from concourse.bass_utils import run_bass_kernel_spmd
    nc = _build_program(reps, matmul_dtype)
    res = run_bass_kernel_spmd(nc, in_maps, core_ids=list(range(N_CORES)))
    return res.results


def kernel(x, segment_pos, w, b):
    in_maps = _make_in_maps(x, segment_pos, w, b)
    results = run_on_device(in_maps)
    return _assemble(results)


# revision 10
# speedup vs baseline: 108.6796x; 1.0085x over previous
"""Trainium2 Bass kernel for masked causal depthwise Conv1D (prefill path).

Reference computation (fp32):
    doc_mask = (segment_pos != 0)                       # [B, S]
    xm = x * doc_mask[..., None]                        # [B, S, W]
    out[b, t, c] = b[c] + sum_k w[k, c] * xm[b, t-3+k, c]   (causal, TW=4)
    new_cache = xm[:, -3:]                              # [B, 3, W]

Shapes: x [4, 4096, 2560] f32, segment_pos [4, 4096] i32, w [4, 2560] f32,
b [2560] f32.

Strategy (8 NeuronCores):
  Shard by (batch, channel-half): core i <- batch i//2, channels
  [1280*(i%2), 1280*(i%2+1)).  Each core receives its x shard transposed to
  channel-major [1280, 4099] (3 leading zero columns = causal pad), so both
  the DMA loads and stores are contiguous per partition and the conv taps
  become free-dim column offsets.

  Per 128-channel block (10 per core):
    - mask:  one fused DVE scalar_tensor_tensor:  xm = (spos != 0) * x
             (spos broadcast across partitions by one step-0 DMA)
    - taps:  4 PE matmuls accumulating in PSUM: lhsT = diag(w_k) (built on
             device as identity * w_k), rhs = xm[:, j*512+k : +512], so
             psum[c, t] += w_k[c] * xm[c, t+k-3].  float32r streams at
             1 cycle/row (fp32 would be 4x slower).
    - evict: ACT Identity with per-partition bias AP: out = psum + b[c]
  Loads run on the SP HWDGE ring (nc.sync), stores + constants on the ACT
  ring (nc.scalar) so the two directions don't serialize on one FIFO.
  Triple-buffered x/xm/out pools keep DMA, DVE, PE and ACT overlapped.

Host does only layout work (transpose/pad/slice); all arithmetic on the
41.9M-element tensors runs on device.
"""

import numpy as np

B, S, W, TW = 4, 4096, 2560, 4
N_CORES = 8
CH = W // 2          # 1280 channels per core
CBS = CH // 128      # 10 channel blocks per core
PAD = TW - 1         # 3
SP = S + PAD         # 4099 padded time columns
NT = S // 512        # 8 psum tiles of 512 per channel block

# matmul dtype: "f32r" = float32r (full-rate, slightly reduced multiply
# precision), "f32" = native fp32 (4 cycles/row, bit-accurate)
MATMUL_DTYPE = "f32r"

_PROGRAM_CACHE: dict = {}


def _build_program(reps: int = 1, matmul_dtype: str = MATMUL_DTYPE):
    """Build the per-core Bass program. All 8 cores run this SPMD with
    different data. reps>1 repeats the whole compute (for timing)."""
    import concourse.bass as bass
    import concourse.tile as tile
    import concourse.mybir as mybir
    from concourse import bacc

    key = (reps, matmul_dtype)
    if key in _PROGRAM_CACHE:
        return _PROGRAM_CACHE[key]

    f32 = mybir.dt.float32
    i32 = mybir.dt.int32
    mm_dt = mybir.dt.float32r if matmul_dtype == "f32r" else f32

    nc = bacc.Bacc("TRN2", target_bir_lowering=False, debug=False,
                   num_devices=N_CORES)

    xp_d = nc.dram_tensor("xp", [CH, SP], f32, kind="ExternalInput")
    spos_d = nc.dram_tensor("spos", [1, S], i32, kind="ExternalInput")
    # compact weights [p, (cb, k)] + identity; diag matrices built on device
    wcol_d = nc.dram_tensor("wcol", [128, CBS * TW], f32, kind="ExternalInput")
    ident_d = nc.dram_tensor("ident", [128, 128], mm_dt, kind="ExternalInput")
    biasr_d = nc.dram_tensor("biasr", [128, CBS], f32, kind="ExternalInput")
    out_d = nc.dram_tensor("outT", [CH, S], f32, kind="ExternalOutput")
    # cache laid out [p, (cb, PAD)]; host un-permutes
    cache_d = nc.dram_tensor("cacheT", [128, CBS * PAD], f32,
                             kind="ExternalOutput")

    with tile.TileContext(nc) as tc:
        with (
            tc.tile_pool(name="const", bufs=1) as const_pool,
            tc.tile_pool(name="xin", bufs=3) as x_pool,
            tc.tile_pool(name="xmask", bufs=3) as xm_pool,
            tc.tile_pool(name="outp", bufs=3) as out_pool,
            tc.tile_pool(name="psum", bufs=8, space="PSUM") as psum_pool,
        ):
            # --- constants ---
            # mask-gating const first on the load ring
            spos_bc = const_pool.tile([128, SP], i32)
            nc.vector.memset(spos_bc[:, 0:PAD], 1)
            nc.sync.dma_start(spos_bc[:, PAD:SP],
                              spos_d.ap().to_broadcast((128, S)))
            # weights/bias on the (idle at prologue) store ring
            wcol_t = const_pool.tile([128, CBS * TW], f32)
            ident_t = const_pool.tile([128, 128], mm_dt)
            nc.scalar.dma_start(wcol_t[:], wcol_d[:])
            nc.scalar.dma_start(ident_t[:], ident_d[:])
            biasr_t = const_pool.tile([128, CBS], f32)
            nc.scalar.dma_start(biasr_t[:], biasr_d[:])
            # diag(w_k) = identity * w_k  (per-partition scalar multiply)
            wdiag_t = const_pool.tile([128, CBS * TW * 128], mm_dt)
            for idx in range(CBS * TW):
                nc.vector.tensor_scalar_mul(
                    wdiag_t[:, idx * 128:(idx + 1) * 128],
                    ident_t[:], wcol_t[:, idx:idx + 1])

            cache_t = const_pool.tile([128, CBS * PAD], f32)

            for rep in range(reps):
                for cb in range(CBS):
                    xt = x_pool.tile([128, SP], f32)
                    nc.sync.dma_start(xt[:], xp_d[cb * 128:(cb + 1) * 128, :])

                    # xm = (spos != 0) * x   — one fused DVE op; output is
                    # written in the matmul streaming dtype (f32r rounds)
                    xm = xm_pool.tile([128, SP], mm_dt)
                    nc.vector.scalar_tensor_tensor(
                        xm[:], spos_bc[:], 0.0, xt[:],
                        op0=mybir.AluOpType.not_equal,
                        op1=mybir.AluOpType.mult)

                    # exact f32 masked copy of the last TW-1 timesteps
                    nc.vector.scalar_tensor_tensor(
                        cache_t[:, cb * PAD:(cb + 1) * PAD],
                        spos_bc[:, S:SP], 0.0, xt[:, S:SP],
                        op0=mybir.AluOpType.not_equal,
                        op1=mybir.AluOpType.mult)

                    ot = out_pool.tile([128, S], f32)
                    for j in range(NT):
                        ps = psum_pool.tile([128, 512], f32)
                        for k in range(TW):
                            idx = cb * TW + k
                            nc.tensor.matmul(
                                ps[:],
                                wdiag_t[:, idx * 128:(idx + 1) * 128],
                                xm[:, j * 512 + k: j * 512 + k + 512],
                                start=(k == 0), stop=(k == TW - 1))
                        # out = psum + bias[c]  (per-partition bias AP)
                        nc.scalar.activation(
                            ot[:, j * 512:(j + 1) * 512], ps[:],
                            mybir.ActivationFunctionType.Identity,
                            bias=biasr_t[:, cb:cb + 1], scale=1.0)

                    nc.scalar.dma_start(out_d[cb * 128:(cb + 1) * 128, :],
                                        ot[:])
                nc.scalar.dma_start(cache_d[:], cache_t[:])

    nc.compile()
    _PROGRAM_CACHE[key] = nc
    return nc


def _make_in_maps(x, segment_pos, w, b):
    """Host-side sharding: pure layout work (transpose, pad, slice)."""
    x = np.asarray(x, dtype=np.float32)
    segment_pos = np.ascontiguousarray(np.asarray(segment_pos, dtype=np.int32))
    w = np.asarray(w, dtype=np.float32)
    b = np.asarray(b, dtype=np.float32)

    # [B, W, S] channel-major copy once
    xT = np.ascontiguousarray(x.transpose(0, 2, 1))

    ident = np.eye(128, dtype=np.float32)
    halves = []
    for h in range(2):
        # wcol[p, cb*TW + k] = w[k, h*CH + cb*128 + p]
        wc = np.ascontiguousarray(
            w[:, h * CH:(h + 1) * CH].reshape(TW, CBS, 128)
            .transpose(2, 1, 0).reshape(128, CBS * TW))
        br = np.ascontiguousarray(
            b[h * CH:(h + 1) * CH].reshape(CBS, 128).T)
        halves.append((wc, br))

    in_maps = []
    for i in range(N_CORES):
        bi, h = divmod(i, 2)
        xp = np.zeros((CH, SP), dtype=np.float32)
        xp[:, PAD:] = xT[bi, h * CH:(h + 1) * CH, :]
        wc, br = halves[h]
        in_maps.append({
            "xp": xp,
            "spos": segment_pos[bi:bi + 1],
            "wcol": wc,
            "ident": ident,
            "biasr": br,
        })
    return in_maps


def _assemble(results):
    out = np.empty((B, S, W), dtype=np.float32)
    cache = np.empty((B, PAD, W), dtype=np.float32)
    for i, r in enumerate(results):
        bi, h = divmod(i, 2)
        out[bi, :, h * CH:(h + 1) * CH] = r["outT"].T
        # cacheT [128, CBS*PAD] -> [CH, PAD]
        ct = r["cacheT"].reshape(128, CBS, PAD).transpose(1, 0, 2).reshape(CH, PAD)
        cache[bi, :, h * CH:(h + 1) * CH] = ct.T
    return out, cache


def run_on_device(in_maps, reps: int = 1, matmul_dtype: str = MATMUL_DTYPE):
    import time as _time
    from concourse.bass_utils import run_bass_kernel_spmd
    nc = _build_program(reps, matmul_dtype)
    last_exc = None
    for attempt in range(3):
        try:
            res = run_bass_kernel_spmd(nc, in_maps,
                                       core_ids=list(range(N_CORES)))
            return res.results
        except Exception as e:  # transient NRT_EXEC_UNIT_UNRECOVERABLE flakes
            last_exc = e
            _time.sleep(2.0 * (attempt + 1))
    raise last_exc


def kernel(x, segment_pos, w, b):
    in_maps = _make_in_maps(x, segment_pos, w, b)
    results = run_on_device(in_maps)
    return _assemble(results)
